# revision 26
# baseline (speedup 1.0000x reference)
"""MixedMoE Trainium2 kernel: sparse expert routing over 8 NeuronCores.

Reference computation (top-2 of 16 experts, combine weight c[t,e] = softmax
score if e in top-2 else exactly 0):
    emb = embeddings.reshape(T, D)
    experts 0..1 consume x, experts 2..15 consume emb (SwiGLU, inter dim H)
    y[t] = sum_e c[t,e] * expert_e(...)[t]          (c exactly 0 off top-2)
    z = silu(emb @ sW1 + sB1) @ sW2 + sB2           (shared experts, all tokens)
    out = (y + z).reshape(B, S, D)

Because c is exactly zero off the top-2, skipping non-routed (token, expert)
pairs is bitwise-identical to the dense reference: we only drop terms that are
0.0 * finite. The host computes the gate (0.03% of the FLOPs), gathers each
expert's routed tokens, and scatters the expert outputs back.

Sharding (SPMD, one program, per-core data):
  core c holds routed experts {2c, 2c+1}; the host gathers each expert's
  routed tokens (padded to a common capacity C, pad slots have c=0 and a
  pad token index not routed to that expert) into a [D, C] activation block.
  The shared experts are token-sharded: core c computes the full 2048-wide
  shared MLP for tokens [512c, 512c+512) of emb. This removes the x-vs-emb
  asymmetry: the host does all gathering/slicing.

On-device per core (all matmuls in float32r = TF32, 1 cycle/row at N>=256):
  per routed expert: u1/u3 = W1s.T @ btT (PSUM, 8 k-tiles); hT = silu(u1+B1)
  * u3 (ACT+DVE, f32r); then y[t_sub, d] = sum_h hT.T @ W2s, scaled by the
  per-token combine weight c (a per-partition scalar after stage 2).
  shared: hT = silu(sW1s.T @ aT + sB1) (ACT direct to f32r); z = sum over 16
  h-tiles of hT.T @ sW2s.
Outputs (single tensor): rows [0,C) expert A, [C,2C) expert B (both already
scaled by c), [2C, 2C+512) the z slice. Host scatters/concats and adds the
purely linear bias terms (c@B2, sB2) exactly.
"""

import os

import numpy as np

B_DIM, S_DIM, D = 4, 1024, 1024
T = B_DIM * S_DIM  # 4096 tokens
H = 1024  # routed expert inter dim
E = 16
N_CORES = 8
E_LOC = 2  # routed experts per core
SH = 2048  # shared experts inter dim
SH_T = SH // 128  # 16 shared h-tiles
TS = T // N_CORES  # 512 shared tokens per core
HT = H // 128  # 8 h-tiles per routed expert
D_T = D // 128  # 8 k-tiles in D

_CACHED = {}  # C -> compiled nc
LAST_IN_MAPS = None  # kept for external timing/debug harnesses


def _subs_for(n):
    """Split n (multiple of 128, >=256) into moving-dim pieces that are all
    >=256 (fp32r runs 1 cycle/row only at moving size >=256) and <=512."""
    out = []
    while n:
        if n <= 512:
            out.append(n)
            break
        if n == 640:
            out.extend([384, 256])
            break
        out.append(512)
        n -= 512
    return out


def _chunks_for(C):
    """Split capacity C into token chunks of <=1024 (weights re-streamed
    per chunk; C <= 1024 in the typical balanced case -> one chunk)."""
    out = [1024] * (C // 1024)
    if C % 1024:
        out.append(C % 1024)
    return out


def _build(C):
    import concourse.tile as tile
    from concourse import bacc, mybir

    f32 = mybir.dt.float32
    f32r = (
        mybir.dt.float32 if os.environ.get("KERNEL_MM_DT") == "f32"
        else mybir.dt.float32r
    )
    SILU = mybir.ActivationFunctionType.Silu
    MULT = mybir.AluOpType.mult
    ADD = mybir.AluOpType.add
    CT = C // 128  # t-subtiles per routed expert

    nc = bacc.Bacc(trn_type="TRN2")

    # ---- DRAM I/O ----
    bt0_d = nc.dram_tensor("bt0", [D, C], f32r, kind="ExternalInput")
    bt1_d = nc.dram_tensor("bt1", [D, C], f32r, kind="ExternalInput")
    at_d = nc.dram_tensor("at", [D, TS], f32r, kind="ExternalInput")
    # W1/W3 pre-laid-out per (expert, h_tile): [e, ht, p, dt, h] so each
    # [128, 8, 128] SBUF tile is one fully-contiguous DRAM block
    w1_d = nc.dram_tensor("w1", [E_LOC, HT, 128, 8, 128], f32r, kind="ExternalInput")
    w3_d = nc.dram_tensor("w3", [E_LOC, HT, 128, 8, 128], f32r, kind="ExternalInput")
    w2_d = nc.dram_tensor("w2", [E_LOC, H, D], f32r, kind="ExternalInput")
    sw1_d = nc.dram_tensor("sw1", [SH_T, 128, 8, 128], f32r, kind="ExternalInput")
    sw2_d = nc.dram_tensor("sw2", [SH, D], f32r, kind="ExternalInput")
    # combine scalars csc[p, e*CT + ts] = c[token in slot ts*128+p, expert e]
    csc_d = nc.dram_tensor("csc", [128, E_LOC * CT], f32, kind="ExternalInput")
    b1_d = nc.dram_tensor("b1", [128, E_LOC * HT], f32, kind="ExternalInput")
    b3_d = nc.dram_tensor("b3", [128, E_LOC * HT], f32, kind="ExternalInput")
    sb1_d = nc.dram_tensor("sb1", [128, SH_T], f32, kind="ExternalInput")
    out_d = nc.dram_tensor("out", [E_LOC * C + TS, D], f32, kind="ExternalOutput")

    with tile.TileContext(nc) as tc:
        with (
            tc.tile_pool(name="small", bufs=1) as small,
            tc.tile_pool(name="btp", bufs=28) as btp,
            tc.tile_pool(name="w13p", bufs=5) as w13p,
            tc.tile_pool(name="w2p", bufs=17) as w2p,
            tc.tile_pool(name="htp", bufs=18) as htp,
            tc.tile_pool(name="silup", bufs=2) as silup,
            tc.tile_pool(name="yp", bufs=5) as ypool,
            tc.tile_pool(name="ps1", bufs=4, space="PSUM") as ps1,
            tc.tile_pool(name="ps2", bufs=3, space="PSUM") as ps2,
        ):
            csc = small.tile([128, E_LOC * CT], f32)
            b1 = small.tile([128, E_LOC * HT], f32)
            b3 = small.tile([128, E_LOC * HT], f32)
            sb1 = small.tile([128, SH_T], f32)
            first = True

            def load_acts(dram, col0, widths):
                tiles = [[None] * len(widths) for _ in range(D_T)]
                for si, w in enumerate(widths):
                    base = col0 + sum(widths[:si])
                    for dt in range(D_T):
                        t = btp.tile([128, 512], f32r, tag="bt")
                        nc.scalar.dma_start(
                            t[:, :w],
                            dram[dt * 128 : (dt + 1) * 128, base : base + w],
                        )
                        tiles[dt][si] = t
                return tiles

            def smalls_once():
                nc.sync.dma_start(sb1[:], sb1_d[:])
                nc.sync.dma_start(csc[:], csc_d[:])
                nc.sync.dma_start(b1[:], b1_d[:])
                nc.sync.dma_start(b3[:], b3_d[:])

            # ---- routed expert phases ----
            for e in range(E_LOC):
                bt_d = (bt0_d, bt1_d)[e]
                col0 = 0
                for chunk in _chunks_for(C):
                    widths = _subs_for(chunk)
                    bts = load_acts(bt_d, col0, widths)
                    if first:
                        smalls_once()
                        first = False
                    hts = [[None] * len(widths) for _ in range(HT)]
                    w2s = []
                    for ht in range(HT):
                        w1s = w13p.tile([128, 8, 128], f32r, tag="w13")
                        nc.sync.dma_start(w1s[:], w1_d[e, ht])
                        w3s = w13p.tile([128, 8, 128], f32r, tag="w13")
                        nc.sync.dma_start(w3s[:], w3_d[e, ht])
                        for si, w in enumerate(widths):
                            u1 = ps1.tile([128, 512], f32, tag="u")
                            u3 = ps1.tile([128, 512], f32, tag="u")
                            for dt in range(D_T):
                                nc.tensor.matmul(
                                    u1[:, :w], lhsT=w1s[:, dt, :],
                                    rhs=bts[dt][si][:, :w],
                                    start=(dt == 0), stop=(dt == D_T - 1),
                                )
                            for dt in range(D_T):
                                nc.tensor.matmul(
                                    u3[:, :w], lhsT=w3s[:, dt, :],
                                    rhs=bts[dt][si][:, :w],
                                    start=(dt == 0), stop=(dt == D_T - 1),
                                )
                            sil = silup.tile([128, 512], f32, tag="sil")
                            nc.scalar.activation(
                                sil[:, :w], u1[:, :w], SILU,
                                bias=b1[:, e * HT + ht : e * HT + ht + 1],
                            )
                            hx = htp.tile([128, 512], f32r, tag="ht")
                            nc.vector.tensor_mul(hx[:, :w], sil[:, :w], u3[:, :w])
                            hts[ht][si] = hx
                        # W2 for this h-tile on the gpsimd queue, interleaved
                        # through stage 1: spreads HBM load and avoids queuing
                        # behind the W1/W3 FIFO on sync
                        w2t = w2p.tile([128, D], f32r, tag="w2")
                        nc.gpsimd.dma_start(
                            w2t[:], w2_d[e, ht * 128 : (ht + 1) * 128, :]
                        )
                        w2s.append(w2t)

                    # tsub -> (sub index, col offset inside that sub)
                    tmap = []
                    for si, w in enumerate(widths):
                        tmap.extend((si, o) for o in range(0, w, 128))
                    for tsub in range(chunk // 128):
                        si, off = tmap[tsub]
                        g = e * CT + (col0 + tsub * 128) // 128
                        yt = ypool.tile([128, D], f32, tag="y")
                        for dch in range(D // 512):
                            acc = ps2.tile([128, 512], f32, tag="acc")
                            for ht in range(HT):
                                nc.tensor.matmul(
                                    acc[:],
                                    lhsT=hts[ht][si][:, off : off + 128],
                                    rhs=w2s[ht][:, dch * 512 : (dch + 1) * 512],
                                    start=(ht == 0), stop=(ht == HT - 1),
                                )
                            nc.vector.tensor_scalar_mul(
                                yt[:, dch * 512 : (dch + 1) * 512],
                                acc[:], csc[:, g : g + 1],
                            )
                        row = e * C + col0 + tsub * 128
                        nc.gpsimd.dma_start(out_d[row : row + 128, :], yt[:])
                    col0 += chunk

            # ---- shared expert phase (512 tokens, full 2048 hidden) ----
            ats = load_acts(at_d, 0, [512])
            sw2s = []
            hsh = [None] * SH_T
            for ht in range(SH_T):
                w1s = w13p.tile([128, 8, 128], f32r, tag="w13")
                nc.sync.dma_start(w1s[:], sw1_d[ht])
                u1 = ps1.tile([128, 512], f32, tag="u")
                for dt in range(D_T):
                    nc.tensor.matmul(
                        u1[:], lhsT=w1s[:, dt, :], rhs=ats[dt][0][:],
                        start=(dt == 0), stop=(dt == D_T - 1),
                    )
                hx = htp.tile([128, 512], f32r, tag="ht")
                nc.scalar.activation(
                    hx[:], u1[:], SILU, bias=sb1[:, ht : ht + 1]
                )
                hsh[ht] = hx
                w2t = w2p.tile([128, D], f32r, tag="w2")
                nc.gpsimd.dma_start(w2t[:], sw2_d[ht * 128 : (ht + 1) * 128, :])
                sw2s.append(w2t)
            for tsub in range(TS // 128):
                zt = ypool.tile([128, D], f32, tag="y")
                for dch in range(D // 512):
                    acc = ps2.tile([128, 512], f32, tag="acc")
                    for ht in range(SH_T):
                        nc.tensor.matmul(
                            acc[:],
                            lhsT=hsh[ht][:, tsub * 128 : (tsub + 1) * 128],
                            rhs=sw2s[ht][:, dch * 512 : (dch + 1) * 512],
                            start=(ht == 0), stop=(ht == SH_T - 1),
                        )
                    nc.vector.tensor_copy(
                        zt[:, dch * 512 : (dch + 1) * 512], acc[:]
                    )
                row = E_LOC * C + tsub * 128
                nc.gpsimd.dma_start(out_d[row : row + 128, :], zt[:])
    nc.compile()
    return nc


def _tf(a):
    return np.ascontiguousarray(np.asarray(a, dtype=np.float32))


def _host_gate(emb2d, gate_w):
    """Replicates softmax + top-2 combine coefficients of the reference."""
    logits = (emb2d @ gate_w.T).astype(np.float32)
    m = logits.max(axis=-1, keepdims=True)
    ex = np.exp(logits - m)
    scores = ex / ex.sum(axis=-1, keepdims=True)  # fp32 softmax
    idx = np.argsort(-scores, axis=-1, kind="stable")[:, :2]  # jax tie order
    c = np.zeros((T, E), dtype=np.float32)
    np.put_along_axis(c, idx, np.take_along_axis(scores, idx, axis=-1), axis=-1)
    return c


def _w13_layout(w):  # [D, H_sl] -> [ht, p, dt, h] contiguous blocks
    hsl = w.shape[1]
    return np.ascontiguousarray(
        w.reshape(8, 128, hsl // 128, 128).transpose(2, 1, 0, 3)
    )


def kernel(embeddings, x, gate_w, W1, B1, W2, B2, W3, B3, sW1, sB1, sW2, sB2):
    global LAST_IN_MAPS
    from concourse.bass_utils import run_bass_kernel_spmd

    embeddings = _tf(embeddings)
    x = _tf(x)
    gate_w, W1, B1, W2, B2, W3, B3 = map(_tf, (gate_w, W1, B1, W2, B2, W3, B3))
    sW1, sB1, sW2, sB2 = map(_tf, (sW1, sB1, sW2, sB2))

    emb2d = embeddings.reshape(T, D)
    embT = np.ascontiguousarray(emb2d.T)
    xT = np.ascontiguousarray(x.T)
    c = _host_gate(emb2d, gate_w)

    routed = c > 0.0  # [T, E] exact sparsity mask
    loads = routed.sum(axis=0)
    C = int(max(256, -(-int(loads.max()) // 128) * 128))  # round up to 128

    # per-expert gathered token indices, padded with a non-routed token so
    # host scatter-add (unique real indices) stays exact
    idx_all, pad_used = [], []
    for e in range(E):
        idx = np.nonzero(routed[:, e])[0]
        free = np.nonzero(~routed[:, e])[0]
        pad = int(free[0]) if len(free) else 0
        idx_p = np.full(C, pad, dtype=np.int64)
        idx_p[: len(idx)] = idx
        idx_all.append(idx_p)
        pad_used.append(len(idx))

    sw1l = _w13_layout(sW1)
    sb1l = np.ascontiguousarray(sB1.reshape(SH_T, 128).T)

    in_maps = []
    for core in range(N_CORES):
        e0 = 2 * core
        w1l = np.stack([_w13_layout(W1[e0 + i]) for i in range(E_LOC)])
        w3l = np.stack([_w13_layout(W3[e0 + i]) for i in range(E_LOC)])
        w2l = np.ascontiguousarray(W2[e0 : e0 + E_LOC])
        srcT = xT if core == 0 else embT  # experts 0,1 consume x
        bts, cscs = [], []
        for i in range(E_LOC):
            idx = idx_all[e0 + i]
            bts.append(np.ascontiguousarray(srcT[:, idx]))
            cv = c[idx, e0 + i].astype(np.float32)
            cv[pad_used[e0 + i] :] = 0.0
            cscs.append(cv.reshape(C // 128, 128).T)  # [128, CT]
        cscc = np.ascontiguousarray(np.concatenate(cscs, axis=1))
        b1c = np.ascontiguousarray(
            B1[e0 : e0 + E_LOC].reshape(E_LOC, HT, 128).transpose(2, 0, 1).reshape(128, -1)
        )
        b3c = np.ascontiguousarray(
            B3[e0 : e0 + E_LOC].reshape(E_LOC, HT, 128).transpose(2, 0, 1).reshape(128, -1)
        )
        atc = np.ascontiguousarray(embT[:, core * TS : (core + 1) * TS])
        in_maps.append(
            {
                "bt0": bts[0], "bt1": bts[1], "at": atc,
                "w1": w1l, "w3": w3l, "w2": w2l,
                "sw1": sw1l, "sw2": sW2, "csc": cscc,
                "b1": b1c, "b3": b3c, "sb1": sb1l,
            }
        )

    LAST_IN_MAPS = in_maps
    if C not in _CACHED:
        _CACHED[C] = _build(C)
    nc = _CACHED[C]

    res = run_bass_kernel_spmd(nc, in_maps, core_ids=list(range(N_CORES)))

    y = np.zeros((T, D), dtype=np.float32)
    for core in range(N_CORES):
        o = res.results[core]["out"]
        y[core * TS : (core + 1) * TS] += o[E_LOC * C :]  # shared slice
        for i in range(E_LOC):
            # pad rows are exactly zero (c=0) and target a non-routed token
            y[idx_all[2 * core + i]] += o[i * C : (i + 1) * C]
    # host-side exact linear bias terms: sum_e c[t,e]*B2[e,:] and sB2
    y += c @ B2
    y += sB2[None, :]
    return y.reshape(B_DIM, S_DIM, D)


# revision 27
# speedup vs baseline: 1.1401x; 1.1401x over previous
"""MixedMoE Trainium2 kernel: sparse expert routing over 8 NeuronCores.

Reference computation (top-2 of 16 experts, combine weight c[t,e] = softmax
score if e in top-2 else exactly 0):
    emb = embeddings.reshape(T, D)
    experts 0..1 consume x, experts 2..15 consume emb (SwiGLU, inter dim H)
    y[t] = sum_e c[t,e] * expert_e(...)[t]          (c exactly 0 off top-2)
    z = silu(emb @ sW1 + sB1) @ sW2 + sB2           (shared experts, all tokens)
    out = (y + z).reshape(B, S, D)

Because c is exactly zero off the top-2, skipping non-routed (token, expert)
pairs is bitwise-identical to the dense reference: we only drop terms that are
0.0 * finite. The host computes the gate (0.03% of the FLOPs), gathers each
expert's routed tokens, and scatters the expert outputs back.

Sharding (SPMD, one program, per-core data):
  core c holds routed experts {2c, 2c+1}; the host gathers each expert's
  routed tokens (padded to a common capacity C, pad slots have c=0 and a
  pad token index not routed to that expert) into a [D, C] activation block.
  The shared experts are token-sharded: core c computes the full 2048-wide
  shared MLP for tokens [512c, 512c+512) of emb. This removes the x-vs-emb
  asymmetry: the host does all gathering/slicing.

On-device per core (all matmuls in float32r = TF32, 1 cycle/row at N>=256):
  per routed expert: u1/u3 = W1s.T @ btT (PSUM, 8 k-tiles); hT = silu(u1+B1)
  * u3 (ACT+DVE, f32r); then y[t_sub, d] = sum_h hT.T @ W2s, scaled by the
  per-token combine weight c (a per-partition scalar after stage 2).
  shared: hT = silu(sW1s.T @ aT + sB1) (ACT direct to f32r); z = sum over 16
  h-tiles of hT.T @ sW2s.
Outputs (single tensor): rows [0,C) expert A, [C,2C) expert B (both already
scaled by c), [2C, 2C+512) the z slice. Host scatters/concats and adds the
purely linear bias terms (c@B2, sB2) exactly.
"""

import os

import numpy as np

B_DIM, S_DIM, D = 4, 1024, 1024
T = B_DIM * S_DIM  # 4096 tokens
H = 1024  # routed expert inter dim
E = 16
N_CORES = 8
E_LOC = 2  # routed experts per core
SH = 2048  # shared experts inter dim
SH_T = SH // 128  # 16 shared h-tiles
TS = T // N_CORES  # 512 shared tokens per core
HT = H // 128  # 8 h-tiles per routed expert
D_T = D // 128  # 8 k-tiles in D

_CACHED = {}  # C -> compiled nc
LAST_IN_MAPS = None  # kept for external timing/debug harnesses


def _subs_for(n):
    """Split n (multiple of 128, >=256) into moving-dim pieces that are all
    >=256 (fp32r runs 1 cycle/row only at moving size >=256) and <=512."""
    out = []
    while n:
        if n <= 512:
            out.append(n)
            break
        if n == 640:
            out.extend([384, 256])
            break
        out.append(512)
        n -= 512
    return out


def _chunks_for(C):
    """Split capacity C into token chunks of <=1024 (weights re-streamed
    per chunk; C <= 1024 in the typical balanced case -> one chunk)."""
    out = [1024] * (C // 1024)
    if C % 1024:
        out.append(C % 1024)
    return out


def _build(C):
    import concourse.tile as tile
    from concourse import bacc, mybir

    f32 = mybir.dt.float32
    f32r = (
        mybir.dt.float32 if os.environ.get("KERNEL_MM_DT") == "f32"
        else mybir.dt.float32r
    )
    SILU = mybir.ActivationFunctionType.Silu
    MULT = mybir.AluOpType.mult
    ADD = mybir.AluOpType.add
    CT = C // 128  # t-subtiles per routed expert

    nc = bacc.Bacc(trn_type="TRN2")

    # ---- DRAM I/O ----
    bt0_d = nc.dram_tensor("bt0", [D, C], f32r, kind="ExternalInput")
    bt1_d = nc.dram_tensor("bt1", [D, C], f32r, kind="ExternalInput")
    at_d = nc.dram_tensor("at", [D, TS], f32r, kind="ExternalInput")
    # W1/W3 pre-laid-out per (expert, h_tile): [e, ht, p, dt, h] so each
    # [128, 8, 128] SBUF tile is one fully-contiguous DRAM block
    w1_d = nc.dram_tensor("w1", [E_LOC, HT, 128, 8, 128], f32r, kind="ExternalInput")
    w3_d = nc.dram_tensor("w3", [E_LOC, HT, 128, 8, 128], f32r, kind="ExternalInput")
    w2_d = nc.dram_tensor("w2", [E_LOC, H, D], f32r, kind="ExternalInput")
    sw1_d = nc.dram_tensor("sw1", [SH_T, 128, 8, 128], f32r, kind="ExternalInput")
    sw2_d = nc.dram_tensor("sw2", [SH, D], f32r, kind="ExternalInput")
    # combine scalars csc[p, e*CT + ts] = c[token in slot ts*128+p, expert e]
    csc_d = nc.dram_tensor("csc", [128, E_LOC * CT], f32, kind="ExternalInput")
    b1_d = nc.dram_tensor("b1", [128, E_LOC * HT], f32, kind="ExternalInput")
    b3_d = nc.dram_tensor("b3", [128, E_LOC * HT], f32, kind="ExternalInput")
    sb1_d = nc.dram_tensor("sb1", [128, SH_T], f32, kind="ExternalInput")
    out_d = nc.dram_tensor("out", [E_LOC * C + TS, D], f32, kind="ExternalOutput")

    with tile.TileContext(nc) as tc:
        with (
            tc.tile_pool(name="small", bufs=1) as small,
            tc.tile_pool(name="btp", bufs=28) as btp,
            tc.tile_pool(name="w13p", bufs=5) as w13p,
            tc.tile_pool(name="w2p", bufs=17) as w2p,
            tc.tile_pool(name="htp", bufs=18) as htp,
            tc.tile_pool(name="silup", bufs=2) as silup,
            tc.tile_pool(name="yp", bufs=5) as ypool,
            tc.tile_pool(name="ps1", bufs=4, space="PSUM") as ps1,
            tc.tile_pool(name="ps2", bufs=3, space="PSUM") as ps2,
        ):
            csc = small.tile([128, E_LOC * CT], f32)
            b1 = small.tile([128, E_LOC * HT], f32)
            b3 = small.tile([128, E_LOC * HT], f32)
            sb1 = small.tile([128, SH_T], f32)
            first = True

            def load_acts(dram, col0, widths):
                tiles = [[None] * len(widths) for _ in range(D_T)]
                for si, w in enumerate(widths):
                    base = col0 + sum(widths[:si])
                    for dt in range(D_T):
                        t = btp.tile([128, 512], f32r, tag="bt")
                        nc.scalar.dma_start(
                            t[:, :w],
                            dram[dt * 128 : (dt + 1) * 128, base : base + w],
                        )
                        tiles[dt][si] = t
                return tiles

            def smalls_once():
                nc.sync.dma_start(sb1[:], sb1_d[:])
                nc.sync.dma_start(csc[:], csc_d[:])
                nc.sync.dma_start(b1[:], b1_d[:])
                nc.sync.dma_start(b3[:], b3_d[:])

            # ---- routed expert phases ----
            for e in range(E_LOC):
                bt_d = (bt0_d, bt1_d)[e]
                col0 = 0
                for chunk in _chunks_for(C):
                    widths = _subs_for(chunk)
                    bts = load_acts(bt_d, col0, widths)
                    if first:
                        smalls_once()
                        first = False
                    hts = [[None] * len(widths) for _ in range(HT)]
                    w2s = []
                    for ht in range(HT):
                        w1s = w13p.tile([128, 8, 128], f32r, tag="w13")
                        nc.sync.dma_start(w1s[:], w1_d[e, ht])
                        w3s = w13p.tile([128, 8, 128], f32r, tag="w13")
                        nc.sync.dma_start(w3s[:], w3_d[e, ht])
                        for si, w in enumerate(widths):
                            u1 = ps1.tile([128, 512], f32, tag="u")
                            u3 = ps1.tile([128, 512], f32, tag="u")
                            for dt in range(D_T):
                                nc.tensor.matmul(
                                    u1[:, :w], lhsT=w1s[:, dt, :],
                                    rhs=bts[dt][si][:, :w],
                                    start=(dt == 0), stop=(dt == D_T - 1),
                                )
                            for dt in range(D_T):
                                nc.tensor.matmul(
                                    u3[:, :w], lhsT=w3s[:, dt, :],
                                    rhs=bts[dt][si][:, :w],
                                    start=(dt == 0), stop=(dt == D_T - 1),
                                )
                            sil = silup.tile([128, 512], f32, tag="sil")
                            nc.scalar.activation(
                                sil[:, :w], u1[:, :w], SILU,
                                bias=b1[:, e * HT + ht : e * HT + ht + 1],
                            )
                            hx = htp.tile([128, 512], f32r, tag="ht")
                            nc.vector.tensor_mul(hx[:, :w], sil[:, :w], u3[:, :w])
                            hts[ht][si] = hx
                        # W2 for this h-tile, interleaved into the sync FIFO
                        # during stage 1 so it arrives well before stage 2
                        w2t = w2p.tile([128, D], f32r, tag="w2")
                        nc.sync.dma_start(
                            w2t[:], w2_d[e, ht * 128 : (ht + 1) * 128, :]
                        )
                        w2s.append(w2t)

                    # tsub -> (sub index, col offset inside that sub)
                    tmap = []
                    for si, w in enumerate(widths):
                        tmap.extend((si, o) for o in range(0, w, 128))
                    for tsub in range(chunk // 128):
                        si, off = tmap[tsub]
                        g = e * CT + (col0 + tsub * 128) // 128
                        yt = ypool.tile([128, D], f32, tag="y")
                        for dch in range(D // 512):
                            acc = ps2.tile([128, 512], f32, tag="acc")
                            for ht in range(HT):
                                nc.tensor.matmul(
                                    acc[:],
                                    lhsT=hts[ht][si][:, off : off + 128],
                                    rhs=w2s[ht][:, dch * 512 : (dch + 1) * 512],
                                    start=(ht == 0), stop=(ht == HT - 1),
                                )
                            nc.vector.tensor_scalar_mul(
                                yt[:, dch * 512 : (dch + 1) * 512],
                                acc[:], csc[:, g : g + 1],
                            )
                        row = e * C + col0 + tsub * 128
                        nc.gpsimd.dma_start(out_d[row : row + 128, :], yt[:])
                    col0 += chunk

            # ---- shared expert phase (512 tokens, full 2048 hidden) ----
            ats = load_acts(at_d, 0, [512])
            sw2s = []
            hsh = [None] * SH_T
            for ht in range(SH_T):
                w1s = w13p.tile([128, 8, 128], f32r, tag="w13")
                nc.sync.dma_start(w1s[:], sw1_d[ht])
                u1 = ps1.tile([128, 512], f32, tag="u")
                for dt in range(D_T):
                    nc.tensor.matmul(
                        u1[:], lhsT=w1s[:, dt, :], rhs=ats[dt][0][:],
                        start=(dt == 0), stop=(dt == D_T - 1),
                    )
                hx = htp.tile([128, 512], f32r, tag="ht")
                nc.scalar.activation(
                    hx[:], u1[:], SILU, bias=sb1[:, ht : ht + 1]
                )
                hsh[ht] = hx
                w2t = w2p.tile([128, D], f32r, tag="w2")
                nc.sync.dma_start(w2t[:], sw2_d[ht * 128 : (ht + 1) * 128, :])
                sw2s.append(w2t)
            for tsub in range(TS // 128):
                zt = ypool.tile([128, D], f32, tag="y")
                for dch in range(D // 512):
                    acc = ps2.tile([128, 512], f32, tag="acc")
                    for ht in range(SH_T):
                        nc.tensor.matmul(
                            acc[:],
                            lhsT=hsh[ht][:, tsub * 128 : (tsub + 1) * 128],
                            rhs=sw2s[ht][:, dch * 512 : (dch + 1) * 512],
                            start=(ht == 0), stop=(ht == SH_T - 1),
                        )
                    nc.vector.tensor_copy(
                        zt[:, dch * 512 : (dch + 1) * 512], acc[:]
                    )
                row = E_LOC * C + tsub * 128
                nc.gpsimd.dma_start(out_d[row : row + 128, :], zt[:])
    nc.compile()
    return nc


def _tf(a):
    return np.ascontiguousarray(np.asarray(a, dtype=np.float32))


def _host_gate(emb2d, gate_w):
    """Replicates softmax + top-2 combine coefficients of the reference."""
    logits = (emb2d @ gate_w.T).astype(np.float32)
    m = logits.max(axis=-1, keepdims=True)
    ex = np.exp(logits - m)
    scores = ex / ex.sum(axis=-1, keepdims=True)  # fp32 softmax
    idx = np.argsort(-scores, axis=-1, kind="stable")[:, :2]  # jax tie order
    c = np.zeros((T, E), dtype=np.float32)
    np.put_along_axis(c, idx, np.take_along_axis(scores, idx, axis=-1), axis=-1)
    return c


def _w13_layout(w):  # [D, H_sl] -> [ht, p, dt, h] contiguous blocks
    hsl = w.shape[1]
    return np.ascontiguousarray(
        w.reshape(8, 128, hsl // 128, 128).transpose(2, 1, 0, 3)
    )


def kernel(embeddings, x, gate_w, W1, B1, W2, B2, W3, B3, sW1, sB1, sW2, sB2):
    global LAST_IN_MAPS
    from concourse.bass_utils import run_bass_kernel_spmd

    embeddings = _tf(embeddings)
    x = _tf(x)
    gate_w, W1, B1, W2, B2, W3, B3 = map(_tf, (gate_w, W1, B1, W2, B2, W3, B3))
    sW1, sB1, sW2, sB2 = map(_tf, (sW1, sB1, sW2, sB2))

    emb2d = embeddings.reshape(T, D)
    embT = np.ascontiguousarray(emb2d.T)
    xT = np.ascontiguousarray(x.T)
    c = _host_gate(emb2d, gate_w)

    routed = c > 0.0  # [T, E] exact sparsity mask
    loads = routed.sum(axis=0)
    C = int(max(256, -(-int(loads.max()) // 128) * 128))  # round up to 128

    # per-expert gathered token indices, padded with a non-routed token so
    # host scatter-add (unique real indices) stays exact
    idx_all, pad_used = [], []
    for e in range(E):
        idx = np.nonzero(routed[:, e])[0]
        free = np.nonzero(~routed[:, e])[0]
        pad = int(free[0]) if len(free) else 0
        idx_p = np.full(C, pad, dtype=np.int64)
        idx_p[: len(idx)] = idx
        idx_all.append(idx_p)
        pad_used.append(len(idx))

    sw1l = _w13_layout(sW1)
    sb1l = np.ascontiguousarray(sB1.reshape(SH_T, 128).T)

    in_maps = []
    for core in range(N_CORES):
        e0 = 2 * core
        w1l = np.stack([_w13_layout(W1[e0 + i]) for i in range(E_LOC)])
        w3l = np.stack([_w13_layout(W3[e0 + i]) for i in range(E_LOC)])
        w2l = np.ascontiguousarray(W2[e0 : e0 + E_LOC])
        srcT = xT if core == 0 else embT  # experts 0,1 consume x
        bts, cscs = [], []
        for i in range(E_LOC):
            idx = idx_all[e0 + i]
            bts.append(np.ascontiguousarray(srcT[:, idx]))
            cv = c[idx, e0 + i].astype(np.float32)
            cv[pad_used[e0 + i] :] = 0.0
            cscs.append(cv.reshape(C // 128, 128).T)  # [128, CT]
        cscc = np.ascontiguousarray(np.concatenate(cscs, axis=1))
        b1c = np.ascontiguousarray(
            B1[e0 : e0 + E_LOC].reshape(E_LOC, HT, 128).transpose(2, 0, 1).reshape(128, -1)
        )
        b3c = np.ascontiguousarray(
            B3[e0 : e0 + E_LOC].reshape(E_LOC, HT, 128).transpose(2, 0, 1).reshape(128, -1)
        )
        atc = np.ascontiguousarray(embT[:, core * TS : (core + 1) * TS])
        in_maps.append(
            {
                "bt0": bts[0], "bt1": bts[1], "at": atc,
                "w1": w1l, "w3": w3l, "w2": w2l,
                "sw1": sw1l, "sw2": sW2, "csc": cscc,
                "b1": b1c, "b3": b3c, "sb1": sb1l,
            }
        )

    LAST_IN_MAPS = in_maps
    if C not in _CACHED:
        _CACHED[C] = _build(C)
    nc = _CACHED[C]

    res = run_bass_kernel_spmd(nc, in_maps, core_ids=list(range(N_CORES)))

    y = np.zeros((T, D), dtype=np.float32)
    for core in range(N_CORES):
        o = res.results[core]["out"]
        y[core * TS : (core + 1) * TS] += o[E_LOC * C :]  # shared slice
        for i in range(E_LOC):
            # pad rows are exactly zero (c=0) and target a non-routed token
            y[idx_all[2 * core + i]] += o[i * C : (i + 1) * C]
    # host-side exact linear bias terms: sum_e c[t,e]*B2[e,:] and sB2
    y += c @ B2
    y += sB2[None, :]
    return y.reshape(B_DIM, S_DIM, D)
